# revision 1
# baseline (speedup 1.0000x reference)
"""Differential attention kernel for 8 Trainium2 NeuronCores — v3.

- v1 attention structure (per-skt scores + 512-wide exp; pairing hurt).
- merged projection pass (q chunks fused, xT read once).
- per-tag psum bufs: score/proj accumulators get 4 slots, v 2, pv 4.
- weight DMAs split per d-tile, wk issued first, so the first k-projection
  matmul starts ~1us in instead of waiting 12.5us for bulk weight DMA.
"""

import math
import os
import time
from contextlib import ExitStack

import ml_dtypes
import numpy as np

import concourse.bass as bass
from concourse import bacc
import concourse.mybir as mybir
import concourse.tile as tile
from concourse.bass_utils import run_bass_kernel_spmd

B, S, D = 4, 4096, 2048
HD = 128
DV = 256
DVA = DV + 1      # + ones column for row sums
SQ = S // 2
N_CORES = 8
DEPTH = 12
SCALE = HD ** -0.5

DT_P = D // 128   # 16 d-tiles
SKT = S // 128    # 32 key tiles
SC = S // 512     # 8 s-chunks
QC = SQ // 512    # 4 q-chunks
SQT = SQ // 128   # 16 q tiles

BF16 = mybir.dt.bfloat16
F32 = mybir.dt.float32

_cache = {}


def build_nc():
    nc = bacc.Bacc("TRN2", target_bir_lowering=False, debug=False)

    xT_d = nc.declare_dram_parameter("xT", [D, S], BF16, isOutput=False)
    wq_d = nc.declare_dram_parameter("wq", [D, DV], BF16, isOutput=False)
    wk_d = nc.declare_dram_parameter("wk", [D, DV], BF16, isOutput=False)
    wv_d = nc.declare_dram_parameter("wv", [D, DV], BF16, isOutput=False)
    lam_d = nc.declare_dram_parameter("lam", [128, 1], F32, isOutput=False)
    out_d = nc.declare_dram_parameter("out", [SQ, DV], F32, isOutput=True)

    xT = xT_d.ap()
    out = out_d.ap()

    with tile.TileContext(nc) as tc, ExitStack() as ctx:
        singles = ctx.enter_context(tc.tile_pool(name="singles", bufs=1))
        x_pool = ctx.enter_context(tc.tile_pool(name="x", bufs=40))
        e_pool = ctx.enter_context(tc.tile_pool(name="e", bufs=6))
        o_pool = ctx.enter_context(tc.tile_pool(name="o", bufs=4))
        r_pool = ctx.enter_context(tc.tile_pool(name="r", bufs=8))

        # --- resident SBUF tensors; weights DMA'd per d-tile, wk first --
        w_sb = {n: singles.tile([128, DT_P, DV], BF16, tag=f"w_{n}", name=f"w_{n}")
                for n in ("wk", "wq", "wv")}
        w_ap = {"wk": wk_d.ap(), "wq": wq_d.ap(), "wv": wv_d.ap()}
        lam_sb = singles.tile([128, 1], F32, tag="lam")
        nc.sync.dma_start(out=lam_sb, in_=lam_d.ap())

        # chunk-0 x tiles interleaved with wk so the first k matmul can
        # start ~1us in; wv/wq follow (needed later in chunk 0)
        xts0 = []
        for dt in range(DT_P):
            nc.sync.dma_start(
                out=w_sb["wk"][:, dt, :],
                in_=w_ap["wk"][dt * 128:(dt + 1) * 128, :],
            )
            xt = x_pool.tile([128, 512], BF16, tag="xt", name=f"xt0_{dt}")
            nc.sync.dma_start(out=xt, in_=xT[dt * 128:(dt + 1) * 128, 0:512])
            xts0.append(xt)
        for n in ("wv", "wq"):
            for dt in range(DT_P):
                nc.sync.dma_start(
                    out=w_sb[n][:, dt, :],
                    in_=w_ap[n][dt * 128:(dt + 1) * 128, :],
                )

        kT = singles.tile([128, 2, S], BF16, tag="kT")        # [dh, head, sk]
        qT = singles.tile([128, 2, SQ], BF16, tag="qT")       # [dh, head, sq]
        v_aug = singles.tile([128, SKT, DVA], BF16, tag="v")  # [s_row, s_tile, dv+1]
        pv1 = singles.tile([128, SQT, DVA], F32, tag="pv1")   # head-1 PV staging

        nc.vector.memset(v_aug[:, :, DV:DVA], 1.0)            # ones column

        # --- projections: one pass over the 8 s-chunks ------------------
        pctx = ExitStack()
        psum = pctx.enter_context(
            tc.tile_pool(name="psum_proj", bufs=4, space=bass.MemorySpace.PSUM)
        )

        # PE warm-up: junk matmuls fill the initial DMA wait so HAM is at
        # 2.4GHz when the first real projection matmul issues.
        jt = singles.tile([128, 512], BF16, tag="junk")
        nc.vector.memset(jt, 0.0)
        jps = psum.tile([128, 512], F32, tag="big_ps", bufs=4, name="jps")
        for w in range(48):
            nc.tensor.matmul(jps, jt[:, 0:128], jt, start=True, stop=True)
        nc.vector.tensor_copy(jt, jps)
        for sc in range(SC):
            if sc == 0:
                xts = xts0
            else:
                xts = []
                for dt in range(DT_P):
                    xt = x_pool.tile([128, 512], BF16, tag="xt", name=f"xt{sc}_{dt}")
                    nc.sync.dma_start(
                        out=xt,
                        in_=xT[dt * 128:(dt + 1) * 128, sc * 512:(sc + 1) * 512],
                    )
                    xts.append(xt)

            projs = [("wk", kT)] + ([("wq", qT)] if sc < QC else [])
            for wname, dst in projs:
                for h in range(2):
                    ps = psum.tile([128, 512], F32, tag="big_ps", bufs=4,
                                   name=f"ps{sc}{wname}{h}")
                    for dt in range(DT_P):
                        nc.tensor.matmul(
                            ps,
                            w_sb[wname][:, dt, h * HD:(h + 1) * HD],
                            xts[dt],
                            start=(dt == 0),
                            stop=(dt == DT_P - 1),
                        )
                    nc.vector.tensor_copy(dst[:, h, sc * 512:(sc + 1) * 512], ps)
            for i in range(4):
                vps = psum.tile([128, DV], F32, tag="v_ps", bufs=2,
                                name=f"vps{sc}_{i}")
                for dt in range(DT_P):
                    nc.tensor.matmul(
                        vps,
                        xts[dt][:, i * 128:(i + 1) * 128],
                        w_sb["wv"][:, dt, :],
                        start=(dt == 0),
                        stop=(dt == DT_P - 1),
                    )
                nc.vector.tensor_copy(v_aug[:, sc * 4 + i, 0:DV], vps)

        pctx.close()

        # --- attention: head 1 then head 2 ------------------------------
        psum = ctx.enter_context(
            tc.tile_pool(name="psum_att", bufs=4, space=bass.MemorySpace.PSUM)
        )
        psum_pv = ctx.enter_context(
            tc.tile_pool(name="psum_pv", bufs=4, space=bass.MemorySpace.PSUM)
        )
        for h in range(2):
            for qc in range(QC):
                pv_ps = [
                    psum_pv.tile([128, DVA], F32, tag="pv_ps", name=f"pv_ps{i}")
                    for i in range(4)
                ]
                for skt in range(SKT):
                    sps = psum.tile([128, 512], F32, tag="big_ps", bufs=4,
                                    name=f"sps{skt}")
                    nc.tensor.matmul(
                        sps,
                        kT[:, h, skt * 128:(skt + 1) * 128],
                        qT[:, h, qc * 512:(qc + 1) * 512],
                    )
                    et = e_pool.tile([128, 512], BF16, tag="et", name=f"et{skt}")
                    nc.scalar.activation(
                        out=et, in_=sps,
                        func=mybir.ActivationFunctionType.Exp,
                        scale=SCALE,
                    )
                    for i in range(4):
                        nc.tensor.matmul(
                            pv_ps[i],
                            et[:, i * 128:(i + 1) * 128],
                            v_aug[:, skt, :],
                            start=(skt == 0),
                            stop=(skt == SKT - 1),
                        )
                for i in range(4):
                    idx = qc * 4 + i
                    if h == 0:
                        nc.vector.tensor_copy(pv1[:, idx, :], pv_ps[i])
                    else:
                        r1 = r_pool.tile([128, 1], F32, tag="r1", name=f"r1_{idx}")
                        r2 = r_pool.tile([128, 1], F32, tag="r2", name=f"r2_{idx}")
                        nc.vector.reciprocal(r1, pv1[:, idx, DV:DVA])
                        nc.vector.reciprocal(r2, pv_ps[i][:, DV:DVA])
                        r2l = r_pool.tile([128, 1], F32, tag="r2l", name=f"r2l_{idx}")
                        nc.vector.tensor_mul(r2l, r2, lam_sb)
                        o1 = o_pool.tile([128, DV], F32, tag="o1", name=f"o1_{idx}")
                        o2 = o_pool.tile([128, DV], F32, tag="o2", name=f"o2_{idx}")
                        nc.vector.tensor_scalar_mul(o1, pv1[:, idx, 0:DV], r1)
                        nc.vector.tensor_scalar_mul(o2, pv_ps[i][:, 0:DV], r2l)
                        ot = o_pool.tile([128, DV], F32, tag="ot", name=f"ot_{idx}")
                        nc.vector.tensor_sub(ot, o1, o2)
                        nc.sync.dma_start(
                            out=out[idx * 128:(idx + 1) * 128, :], in_=ot
                        )

    nc.compile()
    return nc


def _lam(lambda_q1, lambda_q2, lambda_k1, lambda_k2):
    lam_init = 0.8 - 0.6 * math.exp(-0.3 * DEPTH)
    l1 = math.exp(float(np.sum(lambda_q1.astype(np.float64) * lambda_k1.astype(np.float64))))
    l2 = math.exp(float(np.sum(lambda_q2.astype(np.float64) * lambda_k2.astype(np.float64))))
    return l1 + l2 + lam_init


def kernel(x, WQ, WK, WV, lambda_q1, lambda_q2, lambda_k1, lambda_k2):
    if "nc" not in _cache:
        _cache["nc"] = build_nc()
    nc = _cache["nc"]

    bf = ml_dtypes.bfloat16
    lam = np.full((128, 1), _lam(lambda_q1, lambda_q2, lambda_k1, lambda_k2), np.float32)
    wq = np.ascontiguousarray(WQ, dtype=bf)
    wk = np.ascontiguousarray(WK, dtype=bf)
    wv = np.ascontiguousarray(WV, dtype=bf)

    in_maps = []
    for c in range(N_CORES):
        b, qs = c // 2, (c % 2) * SQ
        xb = x[b] if qs == 0 else np.concatenate([x[b, qs:], x[b, :qs]], axis=0)
        xT = np.ascontiguousarray(xb.T, dtype=bf)
        in_maps.append({"xT": xT, "wq": wq, "wk": wk, "wv": wv, "lam": lam})

    kres = None
    for attempt in range(3):
        try:
            kres = run_bass_kernel_spmd(nc, in_maps, list(range(N_CORES)))
            break
        except (ModuleNotFoundError, ImportError):
            # BASS_TRACE requested but this axon build has no NTFF hook
            os.environ["BASS_NEVER_TRACE"] = "1"
        except Exception:
            if attempt == 2:
                raise
            time.sleep(5)
    if kres is None:
        kres = run_bass_kernel_spmd(nc, in_maps, list(range(N_CORES)))
    _cache["last_results"] = kres
    res = kres.results

    out = np.empty((B, S, DV), np.float32)
    for c in range(N_CORES):
        b, qs = c // 2, (c % 2) * SQ
        out[b, qs:qs + SQ] = res[c]["out"]
    return out



# revision 3
# speedup vs baseline: 1.2066x; 1.2066x over previous
"""Differential attention kernel for 8 Trainium2 NeuronCores — v4.

Sharding: (batch, key-half) per core. Each core projects K/V for its
2048-key half only (dedup vs v3's query-split, which duplicated K/V
across the pair), projects Q for all 4096 queries, and computes partial
PV + row-sum accumulators for both heads over its key half. The host
sums the two partials per batch and applies the differential-softmax
normalization (o1/r1 - lam*o2/r2) in numpy.

Projections run as fp8-e4m3 DoubleRow matmuls (cost 0.5 cycles/row,
256-wide contraction) with a 3-term hi/lo residual split:
  x @ W  ~=  xh@Wh + xl@Wh + xh@Wl      (lo*lo term dropped, ~eps^2)
which keeps bf16-level accuracy (measured 3.9e-3 vs 4.5e-3 all-bf16)
at 0.75x the PE cycles. Scores and PV stay bf16 (fp8 cannot hold the
exp() dynamic range; measured catastrophic).

Host preps x/W hi+lo splits and interleaved DRAM layouts so each weight
is one DMA and each x chunk is two (hi/lo).
"""

import math
import os
import time
from contextlib import ExitStack

import ml_dtypes
import numpy as np

import concourse.bass as bass
from concourse import bacc
import concourse.mybir as mybir
import concourse.tile as tile
from concourse.bass_utils import run_bass_kernel_spmd

B, S, D = 4, 4096, 2048
HD = 128
DV = 256
DVA = DV + 1      # + ones column for row sums
SK = S // 2       # keys per core (key-half)
N_CORES = 8
DEPTH = 12
SCALE = HD ** -0.5
WSC = 64.0        # host-side weight scale before fp8 split

DT_P = D // 128   # 16 d-tiles
DP = DT_P // 2    # 8 d-pairs (DoubleRow contraction = 256)
SC = S // 512     # 8 s-chunks (queries)
KC = SK // 512    # 4 s-chunks that are also key chunks
SKT = SK // 128   # 16 key tiles
QC = S // 512     # 8 attention q-chunks
QT = S // 128     # 32 q tiles

BF16 = mybir.dt.bfloat16
F32 = mybir.dt.float32
FP8 = mybir.dt.float8e4
DR = mybir.MatmulPerfMode.DoubleRow
E4 = ml_dtypes.float8_e4m3

INPUT_NAMES = ("xh", "xl", "wqh", "wql", "wkh", "wkl", "wvh", "wvl")

_cache = {}


def build_nc():
    nc = bacc.Bacc("TRN2", target_bir_lowering=False, debug=False)

    # x split halves, host-arranged as [p, dpair, j, chunk, col] so one
    # (chunk) slice is a 2-descriptor-per-partition DMA.
    xh_d = nc.declare_dram_parameter("xh", [128, DP, 2, SC, 512], FP8, isOutput=False)
    xl_d = nc.declare_dram_parameter("xl", [128, DP, 2, SC, 512], FP8, isOutput=False)
    w_d = {}
    for n in ("wqh", "wql", "wkh", "wkl", "wvh", "wvl"):
        # host-arranged [p, dtile, col]
        w_d[n] = nc.declare_dram_parameter(n, [128, DT_P, DV], FP8, isOutput=False)
    # out[h, p, qtile, dva]: per-(qc, h) DMA is an exact [128, 4, 257] match
    out_d = nc.declare_dram_parameter("out", [2, 128, QT, DVA], F32, isOutput=True)

    out = out_d.ap()

    with tile.TileContext(nc) as tc, ExitStack() as ctx:
        singles = ctx.enter_context(tc.tile_pool(name="singles", bufs=1))
        x_pool = ctx.enter_context(tc.tile_pool(name="x", bufs=6))
        e_pool = ctx.enter_context(tc.tile_pool(name="e", bufs=6))
        o_pool = ctx.enter_context(tc.tile_pool(name="o", bufs=4))

        # --- resident SBUF tensors --------------------------------------
        w_sb = {n: singles.tile([128, DT_P, DV], FP8, tag=f"w_{n}", name=f"w_{n}")
                for n in w_d}
        # wk first so the first K matmul can start early
        for n in ("wkh", "wkl", "wvh", "wvl", "wqh", "wql"):
            nc.sync.dma_start(out=w_sb[n], in_=w_d[n].ap())

        kT = singles.tile([128, 2, SK], BF16, tag="kT")       # [dh, head, sk]
        qT = singles.tile([128, 2, S], BF16, tag="qT")        # [dh, head, sq]
        v_aug = singles.tile([128, SKT, DVA], BF16, tag="v")  # [s_row, s_tile, dv+1]

        # ones column carries the row sums; 64 cancels the weight scale
        nc.vector.memset(v_aug[:, :, DV:DVA], WSC)

        # --- projections: one pass over the 8 s-chunks ------------------
        pctx = ExitStack()
        psum = pctx.enter_context(
            tc.tile_pool(name="psum_proj", bufs=4, space=bass.MemorySpace.PSUM)
        )

        # PE warm-up: junk matmuls during the initial DMA wait so the PE
        # p-state is ramped when the first real matmul issues.
        jt = singles.tile([128, 512], BF16, tag="junk")
        nc.vector.memset(jt, 0.0)
        jps = psum.tile([128, 512], F32, tag="big_ps", bufs=4, name="jps")
        for w in range(12):
            nc.tensor.matmul(jps, jt[:, 0:128], jt, start=True, stop=True)
        nc.vector.tensor_copy(jt, jps)

        def proj_cols(ps, wname, hsl, xh_t, xl_t, col0, ncol):
            """3-term DoubleRow accumulation of one [128, ncol] output."""
            first = True
            for wn, xt in ((wname + "h", xh_t), (wname + "h", xl_t),
                           (wname + "l", xh_t)):
                for dp in range(DP):
                    nc.tensor.matmul(
                        ps,
                        w_sb[wn][:, 2 * dp:2 * dp + 2, hsl],
                        xt[:, dp, :, col0:col0 + ncol],
                        start=first,
                        stop=(wn == wname + "l" and dp == DP - 1),
                        perf_mode=DR,
                    )
                    first = False

        for sc in range(SC):
            xh_t = x_pool.tile([128, DP, 2, 512], FP8, tag="xt", name=f"xh{sc}")
            xl_t = x_pool.tile([128, DP, 2, 512], FP8, tag="xt", name=f"xl{sc}")
            nc.sync.dma_start(out=xh_t, in_=xh_d.ap()[:, :, :, sc, :])
            nc.sync.dma_start(out=xl_t, in_=xl_d.ap()[:, :, :, sc, :])

            projs = ([("wk", kT)] if sc < KC else []) + [("wq", qT)]
            for wname, dst in projs:
                for h in range(2):
                    ps = psum.tile([128, 512], F32, tag="big_ps", bufs=4,
                                   name=f"ps{sc}{wname}{h}")
                    proj_cols(ps, wname, slice(h * HD, (h + 1) * HD),
                              xh_t, xl_t, 0, 512)
                    nc.vector.tensor_copy(dst[:, h, sc * 512:(sc + 1) * 512], ps)
            if sc < KC:
                for i in range(4):
                    vps = psum.tile([128, DV], F32, tag="v_ps", bufs=2,
                                    name=f"vps{sc}_{i}")
                    first = True
                    for wn, xt in (("wvh", xh_t), ("wvh", xl_t), ("wvl", xh_t)):
                        for dp in range(DP):
                            nc.tensor.matmul(
                                vps,
                                xt[:, dp, :, i * 128:(i + 1) * 128],
                                w_sb[wn][:, 2 * dp:2 * dp + 2, :],
                                start=first,
                                stop=(wn == "wvl" and dp == DP - 1),
                                perf_mode=DR,
                            )
                            first = False
                    nc.vector.tensor_copy(v_aug[:, sc * 4 + i, 0:DV], vps)

        pctx.close()

        # --- attention: per (head, q-chunk), partial PV over key half ---
        psum = ctx.enter_context(
            tc.tile_pool(name="psum_att", bufs=4, space=bass.MemorySpace.PSUM)
        )
        psum_pv = ctx.enter_context(
            tc.tile_pool(name="psum_pv", bufs=4, space=bass.MemorySpace.PSUM)
        )
        for h in range(2):
            for qc in range(QC):
                pv_ps = [
                    psum_pv.tile([128, DVA], F32, tag="pv_ps", name=f"pv_ps{i}")
                    for i in range(4)
                ]
                for skt in range(SKT):
                    sps = psum.tile([128, 512], F32, tag="big_ps", bufs=4,
                                    name=f"sps{skt}")
                    nc.tensor.matmul(
                        sps,
                        kT[:, h, skt * 128:(skt + 1) * 128],
                        qT[:, h, qc * 512:(qc + 1) * 512],
                    )
                    et = e_pool.tile([128, 512], BF16, tag="et", name=f"et{skt}")
                    nc.scalar.activation(
                        out=et, in_=sps,
                        func=mybir.ActivationFunctionType.Exp,
                        scale=SCALE / (WSC * WSC),
                    )
                    for i in range(4):
                        nc.tensor.matmul(
                            pv_ps[i],
                            et[:, i * 128:(i + 1) * 128],
                            v_aug[:, skt, :],
                            start=(skt == 0),
                            stop=(skt == SKT - 1),
                        )
                ot = o_pool.tile([128, 4, DVA], F32, tag="ot", name=f"ot{h}{qc}")
                for i in range(4):
                    nc.vector.tensor_copy(ot[:, i, :], pv_ps[i])
                nc.sync.dma_start(
                    out=out[h, :, qc * 4:(qc + 1) * 4, :], in_=ot
                )

    nc.compile()
    return nc


def _lam(lambda_q1, lambda_q2, lambda_k1, lambda_k2):
    lam_init = 0.8 - 0.6 * math.exp(-0.3 * DEPTH)
    l1 = math.exp(float(np.sum(lambda_q1.astype(np.float64) * lambda_k1.astype(np.float64))))
    l2 = math.exp(float(np.sum(lambda_q2.astype(np.float64) * lambda_k2.astype(np.float64))))
    return l1 + l2 + lam_init


def _split_x(xT):
    """xT [D, S] f32 -> (hi, lo) e4m3 in [128, DP, 2, SC, 512] layout."""
    xh = xT.astype(E4)
    xl = (xT - xh.astype(np.float32)).astype(E4)
    out = []
    for a in (xh, xl):
        a = a.reshape(DP, 2, 128, SC, 512).transpose(2, 0, 1, 3, 4)
        out.append(np.ascontiguousarray(a))
    return out


def _split_w(W):
    """W [D, DV] f32 -> (hi, lo) e4m3 in [128, DT_P, DV] layout."""
    Ws = W.astype(np.float32) * WSC
    wh = Ws.astype(E4)
    wl = (Ws - wh.astype(np.float32)).astype(E4)
    out = []
    for a in (wh, wl):
        a = a.reshape(DT_P, 128, DV).transpose(1, 0, 2)
        out.append(np.ascontiguousarray(a))
    return out


def kernel(x, WQ, WK, WV, lambda_q1, lambda_q2, lambda_k1, lambda_k2):
    if "nc" not in _cache:
        _cache["nc"] = build_nc()
    nc = _cache["nc"]

    wqh, wql = _split_w(WQ)
    wkh, wkl = _split_w(WK)
    wvh, wvl = _split_w(WV)
    lam = _lam(lambda_q1, lambda_q2, lambda_k1, lambda_k2)

    in_maps = []
    for c in range(N_CORES):
        b, kh = c // 2, c % 2
        xb = x[b] if kh == 0 else np.concatenate([x[b, SK:], x[b, :SK]], axis=0)
        xT = np.ascontiguousarray(xb.T, dtype=np.float32)
        xh, xl = _split_x(xT)
        in_maps.append({
            "xh": xh, "xl": xl,
            "wqh": wqh, "wql": wql, "wkh": wkh, "wkl": wkl,
            "wvh": wvh, "wvl": wvl,
        })

    kres = None
    for attempt in range(3):
        try:
            kres = run_bass_kernel_spmd(nc, in_maps, list(range(N_CORES)))
            break
        except (ModuleNotFoundError, ImportError):
            # BASS_TRACE requested but this axon build has no NTFF hook
            os.environ["BASS_NEVER_TRACE"] = "1"
        except Exception:
            if attempt == 2:
                raise
            time.sleep(5)
    if kres is None:
        kres = run_bass_kernel_spmd(nc, in_maps, list(range(N_CORES)))
    _cache["last_results"] = kres
    res = kres.results

    out = np.empty((B, S, DV), np.float32)
    for b in range(B):
        # out tensor is [2, 128, QT, DVA]: query index = qt*128 + p
        a0 = res[2 * b]["out"].transpose(0, 2, 1, 3).reshape(2, S, DVA)
        a1 = res[2 * b + 1]["out"].transpose(0, 2, 1, 3).reshape(2, S, DVA)
        a1 = np.concatenate([a1[:, SK:], a1[:, :SK]], axis=1)  # un-rotate
        acc = a0.astype(np.float64) + a1.astype(np.float64)
        o1 = acc[0, :, :DV] / acc[0, :, DV:DVA]
        o2 = acc[1, :, :DV] / acc[1, :, DV:DVA]
        out[b] = (o1 - lam * o2).astype(np.float32)
    return out
